# revision 6
# baseline (speedup 1.0000x reference)
"""Trainium2 Bass kernel for:

    sw[b,h,w] = sigmoid( sum_k sp_q[b,k] * sp_wv[b,k,h,w] )
    out[b,c,h,w] = x[b,c,h,w] * (ch_weight[b,c] + sw[b,h,w])

Shapes: B=2048, C=512, C2=256, H=W=7.  Pure data parallel over B across
8 NeuronCores (256 samples per core).  Per-core layout: partition dim =
samples (128 per tile, 2 tiles), free dim = flattened (c, hw) / (k, hw).

Per core, all fp32:
  pass 1 (DVE): for each hw, dot(q[p,:], wv[p,:,hw]) via
                scalar_tensor_tensor accum_out  -> s_raw[p, hw]
  ACT: sigmoid(s_raw) -> s_sig
  pass 2 (DVE): per 64-channel chunk,
                tmp = ch[p,c] (bcast hw) + s_sig[p,hw] (bcast c)
                out = x * tmp
  DMA: loads on nc.sync (HWDGE ring 0), stores on nc.scalar (HWDGE ring 1).
"""

import numpy as np

import concourse.bass as bass
import concourse.bacc as bacc
import concourse.mybir as mybir
from concourse.tile import TileContext
from concourse.bass_utils import run_bass_kernel_spmd

# Problem shapes (hardcoded; kernel.py must be self-contained).
B, C, C2, H, W = 2048, 512, 256, 7, 7
HW = H * W                      # 49
N_CORES = 8
BL = B // N_CORES               # 256 samples per core
P = 128                         # SBUF partitions
NT = BL // P                    # 2 sample-tiles per core
CCH = 64                        # channels per chunk in pass 2
NCH = C // CCH                  # 8 chunks
FCH = CCH * HW                  # 3136 f32 per partition per chunk

F32 = mybir.dt.float32

_NC_CACHE = {}


def build_bass():
    """Build the per-core Bass program (same program on all 8 cores)."""
    if "nc" in _NC_CACHE:
        return _NC_CACHE["nc"]

    # Bacc (not plain Bass): its compile() runs generate_event_semaphores,
    # which splits multi-sem waits — TRN2 instructions have 1 wait slot.
    nc = bacc.Bacc("TRN2")

    x_d = nc.dram_tensor("x", [BL, C * HW], F32, kind="ExternalInput")
    ch_d = nc.dram_tensor("ch", [BL, C], F32, kind="ExternalInput")
    wv_d = nc.dram_tensor("wv", [BL, C2 * HW], F32, kind="ExternalInput")
    q_d = nc.dram_tensor("q", [BL, C2], F32, kind="ExternalInput")
    out_d = nc.dram_tensor("out", [BL, C * HW], F32, kind="ExternalOutput")

    xt = x_d[:].rearrange("(t p) f -> t p f", p=P)
    cht = ch_d[:].rearrange("(t p) f -> t p f", p=P)
    wvt = wv_d[:].rearrange("(t p) f -> t p f", p=P)
    qt = q_d[:].rearrange("(t p) f -> t p f", p=P)
    outt = out_d[:].rearrange("(t p) f -> t p f", p=P)

    with TileContext(nc) as tc:
        with (
            tc.tile_pool(name="wvp", bufs=1) as wvp,
            tc.tile_pool(name="qp", bufs=2) as qp,
            tc.tile_pool(name="chp", bufs=2) as chp,
            tc.tile_pool(name="sp", bufs=2) as sp,
            tc.tile_pool(name="xp", bufs=3) as xp,
            tc.tile_pool(name="tp", bufs=3) as tp,
            tc.tile_pool(name="scrp", bufs=2) as scrp,
        ):
            for t in range(NT):
                wv_s = wvp.tile([P, C2 * HW], F32)
                q_s = qp.tile([P, C2], F32)
                ch_s = chp.tile([P, C], F32)
                nc.sync.dma_start(out=wv_s[:], in_=wvt[t])
                nc.sync.dma_start(out=q_s[:], in_=qt[t])
                nc.sync.dma_start(out=ch_s[:], in_=cht[t])

                # pass 1: s_raw[p, hw] = sum_k wv[p, k, hw] * q[p, k]
                # The S2S2D2_STT instruction has a single sync-wait slot, so
                # the first pass-1 instruction of a tile (which must wait on
                # BOTH the wv and q DMAs) is a plain TensorTensor multiply
                # (multi-wait capable) + small reduce; the remaining hw
                # positions use the fused STT with accum_out (<=1 wait each).
                s_raw = sp.tile([P, HW], F32, tag="s_raw")
                wv3 = wv_s[:].rearrange("p (k h) -> p k h", k=C2)
                prod0 = scrp.tile([P, C2], F32, tag="prod0")
                nc.vector.tensor_tensor(
                    prod0[:], wv3[:, :, 0], q_s[:], mybir.AluOpType.mult
                )
                nc.vector.tensor_reduce(
                    s_raw[:, 0:1],
                    prod0[:],
                    axis=mybir.AxisListType.X,
                    op=mybir.AluOpType.add,
                )
                for hw in range(1, HW):
                    scr = scrp.tile([P, C2], F32, tag="scr")
                    nc.vector.scalar_tensor_tensor(
                        out=scr[:],
                        in0=wv3[:, :, hw],
                        scalar=0.0,
                        in1=q_s[:],
                        op0=mybir.AluOpType.bypass,
                        op1=mybir.AluOpType.mult,
                        accum_out=s_raw[:, hw : hw + 1],
                    )

                s_sig = sp.tile([P, HW], F32, tag="s_sig")
                nc.scalar.activation(
                    out=s_sig[:],
                    in_=s_raw[:],
                    func=mybir.ActivationFunctionType.Sigmoid,
                )

                # pass 2: out = x * (ch + sig) per 64-channel chunk
                for cc in range(NCH):
                    x_s = xp.tile([P, FCH], F32, tag="x")
                    nc.sync.dma_start(
                        out=x_s[:], in_=xt[t][:, cc * FCH : (cc + 1) * FCH]
                    )
                    tmp = tp.tile([P, FCH], F32, tag="tmp")
                    tmp3 = tmp[:].rearrange("p (c h) -> p c h", c=CCH)
                    ch_b = (
                        ch_s[:, cc * CCH : (cc + 1) * CCH]
                        .unsqueeze(2)
                        .broadcast_to([P, CCH, HW])
                    )
                    s_b = s_sig[:].unsqueeze(1).broadcast_to([P, CCH, HW])
                    nc.vector.tensor_tensor(
                        tmp3, ch_b, s_b, mybir.AluOpType.add
                    )
                    nc.vector.tensor_tensor(
                        tmp[:], tmp[:], x_s[:], mybir.AluOpType.mult
                    )
                    # store on the ACT HWDGE ring so loads/stores interleave
                    nc.scalar.dma_start(
                        out=outt[t][:, cc * FCH : (cc + 1) * FCH], in_=tmp[:]
                    )

    nc.compile()
    _NC_CACHE["nc"] = nc
    return nc


def make_in_maps(x, ch_weight, sp_wv, sp_q):
    """Shard full inputs along batch into 8 per-core input maps."""
    x = np.ascontiguousarray(np.asarray(x, dtype=np.float32)).reshape(B, C * HW)
    ch = np.ascontiguousarray(np.asarray(ch_weight, dtype=np.float32)).reshape(B, C)
    wv = np.ascontiguousarray(np.asarray(sp_wv, dtype=np.float32)).reshape(B, C2 * HW)
    q = np.ascontiguousarray(np.asarray(sp_q, dtype=np.float32)).reshape(B, C2)
    in_maps = []
    for c in range(N_CORES):
        sl = slice(c * BL, (c + 1) * BL)
        in_maps.append({"x": x[sl], "ch": ch[sl], "wv": wv[sl], "q": q[sl]})
    return in_maps


def kernel(x, ch_weight, sp_wv, sp_q):
    nc = build_bass()
    in_maps = make_in_maps(x, ch_weight, sp_wv, sp_q)
    res = run_bass_kernel_spmd(nc, in_maps, core_ids=list(range(N_CORES)))
    outs = [res.results[c]["out"] for c in range(N_CORES)]
    full = np.concatenate(outs, axis=0)  # [B, C*HW]
    return full.reshape(B, C, H, W)


# revision 15
# speedup vs baseline: 29.9111x; 29.9111x over previous
"""Trainium2 Bass kernel for:

    sw[b,h,w] = sigmoid( sum_k sp_q[b,k] * sp_wv[b,k,h,w] )
    out[b,c,h,w] = x[b,c,h,w] * (ch_weight[b,c] + sw[b,h,w])

Shapes: B=2048, C=512, C2=256, H=W=7.  Pure data parallel over B across
8 NeuronCores (256 samples per core).  Per-core layout: partition dim =
samples (128 per tile, 2 tiles), free dim = flattened (c, hw) / (k, hw).

Per core, all fp32:
  pass 1 (DVE): for each hw, dot(q[p,:], wv[p,:,hw]) via
                scalar_tensor_tensor accum_out  -> s_raw[p, hw]
  ACT: sigmoid(s_raw) -> s_sig
  pass 2 (DVE): per 128-channel chunk,
                tmp = ch[p,c] (bcast hw) + s_sig[p,hw] (bcast c)
                out = x * tmp
  DMA: loads on nc.sync (HWDGE ring 0), stores on nc.scalar (HWDGE ring 1).
"""

import numpy as np

import concourse.bacc as bacc
import concourse.mybir as mybir
from concourse.tile import TileContext
from concourse.bass_utils import run_bass_kernel_spmd

# Problem shapes (hardcoded; kernel.py must be self-contained).
B, C, C2, H, W = 2048, 512, 256, 7, 7
HW = H * W                      # 49
N_CORES = 8
BL = B // N_CORES               # 256 samples per core
P = 128                         # SBUF partitions
NT = BL // P                    # 2 sample-tiles per core
CCH = 128                       # channels per chunk in pass 2
NCH = C // CCH                  # 8 chunks
FCH = CCH * HW                  # 3136 f32 per partition per chunk

F32 = mybir.dt.float32

_NC_CACHE = {}


def build_bass(reps=1):
    """Build the per-core Bass program (same program on all 8 cores).

    reps > 1 repeats the whole body (for slope-based timing in bench
    scripts); the graded path uses reps=1.
    """
    if reps in _NC_CACHE:
        return _NC_CACHE[reps]

    # Bacc (not plain Bass): its compile() runs generate_event_semaphores,
    # which splits multi-sem waits — TRN2 instructions have 1 wait slot.
    nc = bacc.Bacc("TRN2")

    x_d = nc.dram_tensor("x", [BL, C * HW], F32, kind="ExternalInput")
    ch_d = nc.dram_tensor("ch", [BL, C], F32, kind="ExternalInput")
    wv_d = nc.dram_tensor("wv", [BL, C2 * HW], F32, kind="ExternalInput")
    q_d = nc.dram_tensor("q", [BL, C2], F32, kind="ExternalInput")
    out_d = nc.dram_tensor("out", [BL, C * HW], F32, kind="ExternalOutput")

    xt = x_d[:].rearrange("(t p) f -> t p f", p=P)
    cht = ch_d[:].rearrange("(t p) f -> t p f", p=P)
    wvt = wv_d[:].rearrange("(t p) f -> t p f", p=P)
    qt = q_d[:].rearrange("(t p) f -> t p f", p=P)
    outt = out_d[:].rearrange("(t p) f -> t p f", p=P)

    with TileContext(nc) as tc:
        with (
            tc.tile_pool(name="wvp", bufs=1) as wvp,
            tc.tile_pool(name="qp", bufs=2) as qp,
            tc.tile_pool(name="chp", bufs=2) as chp,
            tc.tile_pool(name="sp", bufs=2) as sp,
            tc.tile_pool(name="xp", bufs=3) as xp,
            tc.tile_pool(name="tp", bufs=2) as tp,
            tc.tile_pool(name="scrp", bufs=2) as scrp,
        ):
            for t in [t for _ in range(reps) for t in range(NT)]:
                wv_s = wvp.tile([P, C2 * HW], F32)
                q_s = qp.tile([P, C2], F32)
                ch_s = chp.tile([P, C], F32)
                nc.sync.dma_start(out=wv_s[:], in_=wvt[t])
                nc.sync.dma_start(out=q_s[:], in_=qt[t])
                nc.sync.dma_start(out=ch_s[:], in_=cht[t])

                # pass 1: s_raw[p, hw] = sum_k wv[p, k, hw] * q[p, k]
                # The S2S2D2_STT instruction has a single sync-wait slot, so
                # the first pass-1 instruction of a tile (which must wait on
                # BOTH the wv and q DMAs) is a plain TensorTensor multiply
                # (multi-wait capable) + small reduce; the remaining hw
                # positions use the fused STT with accum_out (<=1 wait each).
                s_raw = sp.tile([P, HW], F32, tag="s_raw")
                wv3 = wv_s[:].rearrange("p (k h) -> p k h", k=C2)
                prod0 = scrp.tile([P, C2], F32, tag="prod0")
                nc.vector.tensor_tensor(
                    prod0[:], wv3[:, :, 0], q_s[:], mybir.AluOpType.mult
                )
                nc.vector.tensor_reduce(
                    s_raw[:, 0:1],
                    prod0[:],
                    axis=mybir.AxisListType.X,
                    op=mybir.AluOpType.add,
                )
                for hw in range(1, HW):
                    scr = scrp.tile([P, C2], F32, tag="scr")
                    nc.vector.scalar_tensor_tensor(
                        out=scr[:],
                        in0=wv3[:, :, hw],
                        scalar=0.0,
                        in1=q_s[:],
                        op0=mybir.AluOpType.bypass,
                        op1=mybir.AluOpType.mult,
                        accum_out=s_raw[:, hw : hw + 1],
                    )

                s_sig = sp.tile([P, HW], F32, tag="s_sig")
                nc.scalar.activation(
                    out=s_sig[:],
                    in_=s_raw[:],
                    func=mybir.ActivationFunctionType.Sigmoid,
                )

                # pass 2: out = x * (ch + sig) per 128-channel chunk
                for cc in range(NCH):
                    x_s = xp.tile([P, FCH], F32, tag="x")
                    nc.sync.dma_start(
                        out=x_s[:], in_=xt[t][:, cc * FCH : (cc + 1) * FCH]
                    )
                    tmp = tp.tile([P, FCH], F32, tag="tmp")
                    tmp3 = tmp[:].rearrange("p (c h) -> p c h", c=CCH)
                    ch_b = (
                        ch_s[:, cc * CCH : (cc + 1) * CCH]
                        .unsqueeze(2)
                        .broadcast_to([P, CCH, HW])
                    )
                    s_b = s_sig[:].unsqueeze(1).broadcast_to([P, CCH, HW])
                    nc.vector.tensor_tensor(
                        tmp3, ch_b, s_b, mybir.AluOpType.add
                    )
                    nc.vector.tensor_tensor(
                        tmp[:], tmp[:], x_s[:], mybir.AluOpType.mult
                    )
                    # store on the ACT HWDGE ring so loads/stores interleave
                    nc.scalar.dma_start(
                        out=outt[t][:, cc * FCH : (cc + 1) * FCH], in_=tmp[:]
                    )

    nc.compile()
    _NC_CACHE[reps] = nc
    return nc


def make_in_maps(x, ch_weight, sp_wv, sp_q):
    """Shard full inputs along batch into 8 per-core input maps."""
    x = np.ascontiguousarray(np.asarray(x, dtype=np.float32)).reshape(B, C * HW)
    ch = np.ascontiguousarray(np.asarray(ch_weight, dtype=np.float32)).reshape(B, C)
    wv = np.ascontiguousarray(np.asarray(sp_wv, dtype=np.float32)).reshape(B, C2 * HW)
    q = np.ascontiguousarray(np.asarray(sp_q, dtype=np.float32)).reshape(B, C2)
    in_maps = []
    for c in range(N_CORES):
        sl = slice(c * BL, (c + 1) * BL)
        in_maps.append({"x": x[sl], "ch": ch[sl], "wv": wv[sl], "q": q[sl]})
    return in_maps


def kernel(x, ch_weight, sp_wv, sp_q):
    nc = build_bass()
    in_maps = make_in_maps(x, ch_weight, sp_wv, sp_q)
    res = run_bass_kernel_spmd(nc, in_maps, core_ids=list(range(N_CORES)))
    outs = [res.results[c]["out"] for c in range(N_CORES)]
    full = np.concatenate(outs, axis=0)  # [B, C*HW]
    return full.reshape(B, C, H, W)
